# revision 7
# baseline (speedup 1.0000x reference)
"""Clifford LISTA (nn_CliffordLISTA) Trainium2 Bass kernel.

Math: the Cayley table C[i,j,k] of Cl(3,0) is nonzero only at k = i^j.
The geometric product einsum 'bni,hnj,ijk->bhk' therefore collapses to a
single dense matmul after expanding the multivector weights on the host:

    M[(n,i),(h,k)] = C[i, i^k, k] * W[h, n, i^k]        # [N*8, H*8]
    gp(a, W)_flat  = a_flat @ M                          # [B, H*8]

The whole network is then
    W1y = Y @ M1                                         # loop invariant
    x   = 0
    5x: x = soft_threshold(W1y + x @ M2, lam)
with a per-blade (period-8) soft threshold, i.e. a per-feature-channel
clamp:  soft(z) = z - clamp(z, -lam, +lam).

Distribution: data-parallel over batch B=2048 across 8 cores (256 each).
Weights are replicated; no cross-device communication.

On-device layout (per core): everything is kept feature-on-partition,
batch-on-free ("transposed"), so the recurrence needs zero transposes:
    xT tile m: [128 features, 256 batch]
    out^T[m-tile] = sum_k W[k-tile, m-tile].T @ xT[k-tile]
with W tiles streamed from HBM in bf16 (pre-tiled on host so each
m-strip is one contiguous 128-partition DMA).
"""

import numpy as np
import ml_dtypes

import concourse.bass as bass
import concourse.mybir as mybir
from concourse.tile import TileContext

# ---------------- problem constants (hardcoded per contract) ----------------
DIM = 3
NB = 8                      # blades
B, IN, HID = 2048, 256, 512
N_LAYERS = 5
N_CORES = 8
BL = B // N_CORES           # 256 batch per core
P = 128
K1T = IN * NB // P          # 16 k-tiles for W1y
K2T = HID * NB // P         # 32 k-tiles for W2 layers
MT = HID * NB // P          # 32 output feature tiles

_bf16 = ml_dtypes.bfloat16


def _cayley_table(g):
    d = len(g)
    n = 1 << d
    C = np.zeros((n, n, n), dtype=np.float32)
    for a in range(n):
        for b in range(n):
            aa, cnt = a >> 1, 0
            while aa:
                cnt += bin(aa & b).count("1")
                aa >>= 1
            s = -1.0 if (cnt & 1) else 1.0
            for i in range(d):
                if (a >> i) & 1 and (b >> i) & 1:
                    s *= g[i]
            C[a, b, a ^ b] = s
    return C


_C = _cayley_table([1.0, 1.0, 1.0])
_ii = np.arange(NB)[:, None]
_kk = np.arange(NB)[None, :]
_JM = _ii ^ _kk                      # j = i ^ k
_S = _C[_ii, _JM, _kk]               # sign for (i, k)
_GRADES = np.array([bin(i).count("1") for i in range(NB)])


def _expand(W):
    """[H, N, 8] multivector weights -> dense [N*8, H*8] matmul matrix."""
    Wt = np.ascontiguousarray(W.transpose(1, 0, 2))          # [N, H, 8]
    M = (Wt[:, :, _JM] * _S[None, None]).transpose(0, 2, 1, 3)
    n, h = W.shape[1], W.shape[0]
    return np.ascontiguousarray(M).reshape(n * NB, h * NB)


def _tile_weights(M, kt):
    """[K, 4096] -> [32 m-strips, 128 partitions, kt*128] bf16, each strip
    contiguous so one dma_start moves it at line rate."""
    K = kt * P
    T = M.reshape(kt, P, MT, P).transpose(2, 1, 0, 3)        # [mo, p, ko, mi]
    return np.ascontiguousarray(T.astype(_bf16)).reshape(MT, P, kt * P)


def _build_program():
    dt = mybir.dt
    nc = bass.Bass()

    y_d = nc.declare_dram_parameter("y", [P, K1T * BL], dt.bfloat16, isOutput=False)
    w1_d = nc.declare_dram_parameter("w1", [MT, P, K1T * P], dt.bfloat16, isOutput=False)
    w2_d = nc.declare_dram_parameter("w2", [MT, P, K2T * P], dt.bfloat16, isOutput=False)
    lam_d = nc.declare_dram_parameter("lam", [P, 2], dt.float32, isOutput=False)
    out_d = nc.declare_dram_parameter("out", [MT, P, BL], dt.float32, isOutput=True)

    with TileContext(nc) as tc:
        with (
            tc.tile_pool(name="const", bufs=1) as constp,
            tc.tile_pool(name="state", bufs=1) as statep,
            tc.tile_pool(name="wpool", bufs=4) as wpool,
            tc.tile_pool(name="psum", bufs=8, space="PSUM") as ppool,
            tc.tile_pool(name="work", bufs=4) as workp,
        ):
            lam2 = constp.tile([P, 2], dt.float32, tag="lam2")
            nc.sync.dma_start(out=lam2[:], in_=lam_d[:])
            lamp = lam2[:, 0:1]
            lamn = lam2[:, 1:2]
            ysb = constp.tile([P, K1T * BL], dt.bfloat16, tag="y")
            nc.sync.dma_start(out=ysb[:], in_=y_d[:])

            w1y = statep.tile([P, MT * BL], dt.float32, tag="w1y")
            xa = statep.tile([P, MT * BL], dt.bfloat16, tag="xa")
            xb = statep.tile([P, MT * BL], dt.bfloat16, tag="xb")
            xbufs = [xa, xb]

            # ---- phase 0: W1y, save it, and x1 = soft(W1y) -> xa ----
            for m in range(MT):
                w = wpool.tile([P, K1T * P], dt.bfloat16, tag="w1s")
                nc.sync.dma_start(out=w[:], in_=w1_d[m])
                ps = ppool.tile([P, BL], dt.float32, tag="ps")
                for k in range(K1T):
                    nc.tensor.matmul(
                        ps[:],
                        lhsT=w[:, k * P:(k + 1) * P],
                        rhs=ysb[:, k * BL:(k + 1) * BL],
                        start=(k == 0),
                        stop=(k == K1T - 1),
                    )
                zs = w1y[:, m * BL:(m + 1) * BL]
                nc.vector.tensor_copy(zs, ps[:])                 # save W1y (f32)
                c = workp.tile([P, BL], dt.float32, tag="c")
                nc.vector.tensor_scalar(
                    c[:], zs, lamp, lamn,
                    op0=mybir.AluOpType.min, op1=mybir.AluOpType.max,
                )
                nc.vector.tensor_sub(xa[:, m * BL:(m + 1) * BL], zs, c[:])

            # ---- 4 refinement layers: x <- soft(W1y + x @ M2) ----
            for it in range(N_LAYERS - 1):
                x_cur = xbufs[it % 2]
                x_next = xbufs[(it + 1) % 2]
                last = it == N_LAYERS - 2
                for m in range(MT):
                    w = wpool.tile([P, K2T * P], dt.bfloat16, tag="w2s")
                    nc.sync.dma_start(out=w[:], in_=w2_d[m])
                    ps = ppool.tile([P, BL], dt.float32, tag="ps")
                    for k in range(K2T):
                        nc.tensor.matmul(
                            ps[:],
                            lhsT=w[:, k * P:(k + 1) * P],
                            rhs=x_cur[:, k * BL:(k + 1) * BL],
                            start=(k == 0),
                            stop=(k == K2T - 1),
                        )
                    z = workp.tile([P, BL], dt.float32, tag="z")
                    nc.vector.tensor_add(z[:], ps[:], w1y[:, m * BL:(m + 1) * BL])
                    c = workp.tile([P, BL], dt.float32, tag="c")
                    nc.vector.tensor_scalar(
                        c[:], z[:], lamp, lamn,
                        op0=mybir.AluOpType.min, op1=mybir.AluOpType.max,
                    )
                    if last:
                        xo = workp.tile([P, BL], dt.float32, tag="xo")
                        nc.vector.tensor_sub(xo[:], z[:], c[:])
                        nc.sync.dma_start(out=out_d[m], in_=xo[:])
                    else:
                        nc.vector.tensor_sub(
                            x_next[:, m * BL:(m + 1) * BL], z[:], c[:]
                        )
    return nc


def _split_multi_waits(m):
    """The walrus in this image packs exactly one sync-wait slot per ISA
    instruction; Tile emits several. Hoist the extras onto standalone
    EventSemaphore instructions on the same engine immediately before the
    instruction (identical semantics: all waits gate the same program point).
    """
    for f in m.functions:
        for blk in f.blocks:
            out = []
            for inst in blk.instructions:
                si = inst.sync_info
                if si is not None and si.on_wait is not None and len(si.on_wait) > 1:
                    waits = list(si.on_wait)
                    for j, w in enumerate(waits[:-1]):
                        out.append(
                            mybir.InstEventSemaphore(
                                name=f"{inst.name}-w{j}",
                                opcode="EventSemaphore",
                                engine=inst.engine,
                                ins=[],
                                outs=[],
                                sync_info=mybir.SyncInfo(on_wait=[w], on_update=[]),
                            )
                        )
                    si.on_wait = [waits[-1]]
                out.append(inst)
            blk.instructions = out


_CACHE = {}


def _prep_inputs(y, W1, W2, lambdas):
    M1 = _expand(np.asarray(W1, dtype=np.float32))           # [2048, 4096]
    M2 = _expand(np.asarray(W2, dtype=np.float32))           # [4096, 4096]
    w1_t = _tile_weights(M1, K1T)
    w2_t = _tile_weights(M2, K2T)

    lam_blade = np.asarray(lambdas, dtype=np.float32)[_GRADES]   # [8]
    lam_part = np.tile(lam_blade, P // NB).astype(np.float32)
    lam2 = np.ascontiguousarray(np.stack([lam_part, -lam_part], axis=1))

    Y = np.asarray(y, dtype=np.float32).reshape(B, IN * NB)
    in_maps = []
    for cid in range(N_CORES):
        Yc = Y[cid * BL:(cid + 1) * BL]                      # [256, 2048]
        yT = Yc.T.reshape(K1T, P, BL).transpose(1, 0, 2)     # [128, 16, 256]
        yT = np.ascontiguousarray(yT.astype(_bf16)).reshape(P, K1T * BL)
        in_maps.append({"y": yT, "w1": w1_t, "w2": w2_t, "lam": lam2})
    return in_maps


def _gather(results):
    X = np.empty((B, HID * NB), dtype=np.float32)
    for cid in range(N_CORES):
        o = results[cid]["out"]                              # [32, 128, 256]
        X[cid * BL:(cid + 1) * BL] = o.transpose(2, 0, 1).reshape(BL, HID * NB)
    return X.reshape(B, HID, NB)


def _get_exec():
    """Compile (once) and return the sharded PJRT executable for the program.

    Mirrors concourse.bass2jax.run_bass_via_pjrt's multi-core path but keeps
    the jitted callable so repeated executions don't re-trace/re-compile.
    """
    if "exec" in _CACHE:
        return _CACHE["exec"]
    import jax
    from concourse import bass2jax as b2j

    nc = _build_program()
    _split_multi_waits(nc.m)
    assert nc.dbg_addr is None
    partition_name = nc.partition_id_tensor.name if nc.partition_id_tensor else None

    b2j.install_neuronx_cc_hook()
    in_names, out_names, out_avals = [], [], []
    for alloc in nc.m.functions[0].allocations:
        if not isinstance(alloc, mybir.MemoryLocationSet):
            continue
        name = alloc.memorylocations[0].name
        if alloc.kind == "ExternalInput":
            if name != partition_name:
                in_names.append(name)
        elif alloc.kind == "ExternalOutput":
            out_names.append(name)
            out_avals.append(
                jax.core.ShapedArray(tuple(alloc.tensor_shape), mybir.dt.np(alloc.dtype))
            )
    n_params, n_outs = len(in_names), len(out_names)
    all_in_names = tuple(in_names + out_names)
    if partition_name is not None:
        all_in_names = all_in_names + (partition_name,)

    def _body(*args):
        operands = list(args)
        if partition_name is not None:
            operands.append(b2j.partition_id_tensor())
        return tuple(
            b2j._bass_exec_p.bind(
                *operands,
                out_avals=tuple(out_avals),
                in_names=all_in_names,
                out_names=tuple(out_names),
                lowering_input_output_aliases=(),
                sim_require_finite=True,
                sim_require_nnan=True,
                nc=nc,
            )
        )

    devices = jax.devices()[:N_CORES]
    assert len(devices) == N_CORES
    mesh = b2j.Mesh(np.asarray(devices), ("core",))
    in_specs = (b2j.PartitionSpec("core"),) * (n_params + n_outs)
    out_specs = (b2j.PartitionSpec("core"),) * n_outs
    donate = tuple(range(n_params, n_params + n_outs))
    sharded = jax.jit(
        b2j.shard_map(
            _body, mesh=mesh, in_specs=in_specs, out_specs=out_specs, check_rep=False
        ),
        donate_argnums=donate,
        keep_unused=True,
    )
    _CACHE["exec"] = (sharded, in_names, out_names, out_avals, mesh)
    return _CACHE["exec"]


def _stage(y, W1, W2, lambdas):
    """Host prep + device staging. Returns (sharded_fn, dev_inputs, zero_outs)."""
    import jax
    from jax.sharding import NamedSharding, PartitionSpec

    sharded, in_names, out_names, out_avals, mesh = _get_exec()
    in_maps = _prep_inputs(y, W1, W2, lambdas)
    concat_in = [
        np.concatenate([in_maps[c][name] for c in range(N_CORES)], axis=0)
        for name in in_names
    ]
    sh = NamedSharding(mesh, PartitionSpec("core"))
    dev_in = [jax.device_put(a, sh) for a in concat_in]
    zeros = [
        jax.device_put(
            np.zeros((N_CORES * av.shape[0], *av.shape[1:]), av.dtype), sh
        )
        for av in out_avals
    ]
    return sharded, dev_in, zeros, out_avals


def _run(y, W1, W2, lambdas):
    sharded, dev_in, zeros, out_avals = _stage(y, W1, W2, lambdas)
    outs = sharded(*dev_in, *zeros)
    o = np.asarray(outs[0]).reshape(N_CORES, *out_avals[0].shape)
    return _gather([{"out": o[c]} for c in range(N_CORES)])


def kernel(y, W1, W2, lambdas):
    return _run(y, W1, W2, lambdas)


# revision 20
# speedup vs baseline: 1.1368x; 1.1368x over previous
"""Clifford LISTA (nn_CliffordLISTA) Trainium2 Bass kernel.

Math: the Cayley table C[i,j,k] of Cl(3,0) is nonzero only at k = i^j, so
the geometric product einsum 'bni,hnj,ijk->bhk' is, per output blade k:

    out[b,h,k] = sum_{n,i} x[b,n,i] * W[h,n,i^k] * S(i,k),   S(i,k)=C[i,i^k,k]

i.e. each output blade contracts the SAME raw weight tensor W (4MB bf16 for
W2 — it stays resident in SBUF) against x, with a per-(i,k) sign +-1 and a
blade permutation j=i^k that are handled by slice selection at trace time.
The sign is folded into the rhs by keeping a negated copy of x (computed for
free in the epilogue: z-c and c-z).

Feature layout is blade-major (f = blade*H + h) on device so the recurrence
x -> soft(W1y + W2 x) needs zero transposes: xT tiles are
[128 features(part), 256 batch(free)], contraction index is (i, n) blade-
major, and the matmul out^T tile for (blade k, h-chunk) accumulates 32
k-tiles: lhsT = W2sb[:, (j=i^k, nc, hc)-slice], rhs = (+-x)[:, (i,nc)-slice].

Per-blade soft threshold = per-feature clamp: soft(z) = z - clamp(z,-l,+l);
blade is constant within an output tile, so lambda is a [128,1] AP per tile.

Distribution: data-parallel over batch B=2048 across 8 cores (256 each),
weights replicated. HBM traffic per core is only ~11MB total.
"""

import numpy as np
import ml_dtypes

import concourse.bass as bass
import concourse.mybir as mybir
from concourse.tile import TileContext

# ---------------- problem constants (hardcoded per contract) ----------------
DIM = 3
NB = 8                      # blades
B, IN, HID = 2048, 256, 512
N_LAYERS = 5
N_CORES = 8
BL = B // N_CORES           # 256 batch per core
P = 128
NC1 = IN // P               # 2 n-chunks for W1y contraction
NC2 = HID // P              # 4 n-chunks for W2 contraction
K1T = NB * NC1              # 16 k-tiles for W1y
K2T = NB * NC2              # 32 k-tiles for W2 layers
HC = HID // P               # 4 h-chunks per blade
MT = NB * HC                # 32 output feature tiles (blade-major)

_bf16 = ml_dtypes.bfloat16


def _cayley_table(g):
    d = len(g)
    n = 1 << d
    C = np.zeros((n, n, n), dtype=np.float32)
    for a in range(n):
        for b in range(n):
            aa, cnt = a >> 1, 0
            while aa:
                cnt += bin(aa & b).count("1")
                aa >>= 1
            s = -1.0 if (cnt & 1) else 1.0
            for i in range(d):
                if (a >> i) & 1 and (b >> i) & 1:
                    s *= g[i]
            C[a, b, a ^ b] = s
    return C


_C = _cayley_table([1.0, 1.0, 1.0])
_ii = np.arange(NB)[:, None]
_kk = np.arange(NB)[None, :]
_S = _C[_ii, _ii ^ _kk, _kk]         # S[i,k] = C[i, i^k, k], all +-1
_GRADES = np.array([bin(i).count("1") for i in range(NB)])


def _pack_weights(W):
    """[H, N, 8] -> [128, 8*nch*H] bf16, partition-major: per partition p the
    cols are (j, nc, h) blocks with value W[h, nc*128+p, j]."""
    H, N = W.shape[0], W.shape[1]
    ncn = N // P
    T = W.transpose(2, 1, 0).reshape(NB, ncn, P, H).transpose(2, 0, 1, 3)
    return np.ascontiguousarray(T.astype(_bf16)).reshape(P, NB * ncn * H)


def _build_program(variant="full", reps=1):
    dt = mybir.dt
    nc = bass.Bass()

    y_d = nc.declare_dram_parameter("y", [P, K1T * BL], dt.bfloat16, isOutput=False)
    w1_d = nc.declare_dram_parameter("w1", [P, NB * NC1 * HID], dt.bfloat16, isOutput=False)
    w2_d = nc.declare_dram_parameter("w2", [P, NB * NC2 * HID], dt.bfloat16, isOutput=False)
    lam_d = nc.declare_dram_parameter("lam", [P, 2 * NB], dt.float32, isOutput=False)
    out_d = nc.declare_dram_parameter("out", [MT, P, BL], dt.float32, isOutput=True)

    with TileContext(nc) as tc:
        with (
            tc.tile_pool(name="const", bufs=1) as constp,
            tc.tile_pool(name="state", bufs=1) as statep,
            tc.tile_pool(name="psum", bufs=8, space="PSUM") as ppool,
            tc.tile_pool(name="work", bufs=4) as workp,
        ):
            lam2 = constp.tile([P, 2 * NB], dt.float32, tag="lam2")
            nc.sync.dma_start(out=lam2[:], in_=lam_d[:])
            w2sb = constp.tile([P, NB * NC2 * HID], dt.bfloat16, tag="w2")
            nc.sync.dma_start(out=w2sb[:], in_=w2_d[:])
            w1sb = constp.tile([P, NB * NC1 * HID], dt.bfloat16, tag="w1")
            nc.scalar.dma_start(out=w1sb[:], in_=w1_d[:])
            ysb = constp.tile([P, K1T * BL], dt.bfloat16, tag="y")
            nc.scalar.dma_start(out=ysb[:], in_=y_d[:])
            # negated y for the sign-folded contraction
            ynsb = constp.tile([P, K1T * BL], dt.bfloat16, tag="yn")
            nc.vector.tensor_scalar_mul(ynsb[:], ysb[:], -1.0)

            w1y = statep.tile([P, MT * BL], dt.float32, tag="w1y")
            xp = [
                statep.tile([P, MT * BL], dt.bfloat16, tag=f"x{n}", name=f"x{n}")
                for n in range(2)
            ]
            xn = [
                statep.tile([P, MT * BL], dt.bfloat16, tag=f"xn{n}", name=f"xn{n}")
                for n in range(2)
            ]

            def w1slice(j, ncn, hc):
                base = (j * NC1 + ncn) * HID + hc * P
                return w1sb[:, base:base + P]

            def w2slice(j, ncn, hc):
                base = (j * NC2 + ncn) * HID + hc * P
                return w2sb[:, base:base + P]

            for rep in range(reps):
                # ---- phase 0: W1y (blade-major), save, x1 = soft(W1y) ----
                for k in range(NB):
                    lamp = lam2[:, 2 * k:2 * k + 1]
                    lamn = lam2[:, 2 * k + 1:2 * k + 2]
                    for hc in range(HC):
                        m = k * HC + hc
                        ps = ppool.tile([P, BL], dt.float32, tag="ps")
                        t = 0
                        for i in range(NB):
                            j = i ^ k
                            pos = _S[i, k] > 0
                            for ncn in range(NC1):
                                rhs_t = ysb if pos else ynsb
                                kt = i * NC1 + ncn
                                nc.tensor.matmul(
                                    ps[:],
                                    lhsT=w1slice(j, ncn, hc),
                                    rhs=rhs_t[:, kt * BL:(kt + 1) * BL],
                                    start=(t == 0),
                                    stop=(t == K1T - 1),
                                )
                                t += 1
                        zs = w1y[:, m * BL:(m + 1) * BL]
                        nc.vector.tensor_copy(zs, ps[:])
                        c = workp.tile([P, BL], dt.float32, tag="c")
                        nc.vector.tensor_scalar(
                            c[:], zs, lamp, lamn,
                            op0=mybir.AluOpType.min, op1=mybir.AluOpType.max,
                        )
                        sl = slice(m * BL, (m + 1) * BL)
                        nc.vector.tensor_sub(xp[0][:, sl], zs, c[:])
                        nc.vector.tensor_sub(xn[0][:, sl], c[:], zs)

                # ---- 4 refinement layers ----
                for it in range(N_LAYERS - 1):
                    cur, nxt = it % 2, (it + 1) % 2
                    last = it == N_LAYERS - 2
                    for k in range(NB):
                        lamp = lam2[:, 2 * k:2 * k + 1]
                        lamn = lam2[:, 2 * k + 1:2 * k + 2]
                        for hc in range(HC):
                            m = k * HC + hc
                            ps = ppool.tile([P, BL], dt.float32, tag="ps")
                            t = 0
                            for i in range(NB):
                                j = i ^ k
                                pos = _S[i, k] > 0
                                for ncn in range(NC2):
                                    rhs_t = xp[cur] if pos else xn[cur]
                                    kt = i * NC2 + ncn
                                    nc.tensor.matmul(
                                        ps[:],
                                        lhsT=w2slice(j, ncn, hc),
                                        rhs=rhs_t[:, kt * BL:(kt + 1) * BL],
                                        start=(t == 0),
                                        stop=(t == K2T - 1),
                                    )
                                    t += 1
                            z = workp.tile([P, BL], dt.float32, tag="z")
                            nc.vector.tensor_add(z[:], ps[:], w1y[:, m * BL:(m + 1) * BL])
                            c = workp.tile([P, BL], dt.float32, tag="c")
                            nc.vector.tensor_scalar(
                                c[:], z[:], lamp, lamn,
                                op0=mybir.AluOpType.min, op1=mybir.AluOpType.max,
                            )
                            if last:
                                xo = workp.tile([P, BL], dt.float32, tag="xo")
                                nc.vector.tensor_sub(xo[:], z[:], c[:])
                                nc.sync.dma_start(out=out_d[m], in_=xo[:])
                            else:
                                sl = slice(m * BL, (m + 1) * BL)
                                nc.vector.tensor_sub(xp[nxt][:, sl], z[:], c[:])
                                nc.vector.tensor_sub(xn[nxt][:, sl], c[:], z[:])
    return nc


def _split_multi_waits(m):
    """The walrus in this image packs exactly one sync-wait slot per ISA
    instruction; Tile emits several. Hoist the extras onto standalone
    EventSemaphore instructions on the same engine immediately before the
    instruction (identical semantics: all waits gate the same program point).
    """
    for f in m.functions:
        for blk in f.blocks:
            out = []
            for inst in blk.instructions:
                si = inst.sync_info
                if si is not None and si.on_wait is not None and len(si.on_wait) > 1:
                    waits = list(si.on_wait)
                    for j, w in enumerate(waits[:-1]):
                        out.append(
                            mybir.InstEventSemaphore(
                                name=f"{inst.name}-w{j}",
                                opcode="EventSemaphore",
                                engine=inst.engine,
                                ins=[],
                                outs=[],
                                sync_info=mybir.SyncInfo(on_wait=[w], on_update=[]),
                            )
                        )
                    si.on_wait = [waits[-1]]
                out.append(inst)
            blk.instructions = out


_CACHE = {}


def _prep_inputs(y, W1, W2, lambdas):
    w1_t = _pack_weights(np.asarray(W1, dtype=np.float32))   # [128, 8192]
    w2_t = _pack_weights(np.asarray(W2, dtype=np.float32))   # [128, 16384]

    lam_blade = np.asarray(lambdas, dtype=np.float32)[_GRADES]   # [8]
    lam2 = np.zeros((P, 2 * NB), np.float32)
    lam2[:, 0::2] = lam_blade[None, :]
    lam2[:, 1::2] = -lam_blade[None, :]

    # y -> blade-major transposed tiles: part p of k-tile (i,ncn) is
    # y[b, ncn*128+p, i]
    Y = np.asarray(y, dtype=np.float32)                      # [B, IN, 8]
    in_maps = []
    for cid in range(N_CORES):
        Yc = Y[cid * BL:(cid + 1) * BL]                      # [256, 256, 8]
        yT = Yc.transpose(2, 1, 0).reshape(NB, NC1, P, BL)   # [i, nc, p, b]
        yT = yT.transpose(2, 0, 1, 3).reshape(P, K1T * BL)   # [p, (i,nc,b)]
        yT = np.ascontiguousarray(yT.astype(_bf16))
        in_maps.append({"y": yT, "w1": w1_t, "w2": w2_t, "lam": lam2})
    return in_maps


def _gather(results):
    X = np.empty((B, NB * HID), dtype=np.float32)
    for cid in range(N_CORES):
        o = results[cid]["out"]                              # [32, 128, 256]
        X[cid * BL:(cid + 1) * BL] = o.transpose(2, 0, 1).reshape(BL, NB * HID)
    # blade-major f = k*HID + h  ->  [B, H, blades]
    return np.ascontiguousarray(X.reshape(B, NB, HID).transpose(0, 2, 1))


def _get_exec():
    """Compile (once) and return the sharded PJRT executable for the program.

    Mirrors concourse.bass2jax.run_bass_via_pjrt's multi-core path but keeps
    the jitted callable so repeated executions don't re-trace/re-compile.
    """
    if "exec" in _CACHE:
        return _CACHE["exec"]
    import jax
    from concourse import bass2jax as b2j

    nc = _build_program()
    _split_multi_waits(nc.m)
    assert nc.dbg_addr is None
    partition_name = nc.partition_id_tensor.name if nc.partition_id_tensor else None

    b2j.install_neuronx_cc_hook()
    in_names, out_names, out_avals = [], [], []
    for alloc in nc.m.functions[0].allocations:
        if not isinstance(alloc, mybir.MemoryLocationSet):
            continue
        name = alloc.memorylocations[0].name
        if alloc.kind == "ExternalInput":
            if name != partition_name:
                in_names.append(name)
        elif alloc.kind == "ExternalOutput":
            out_names.append(name)
            out_avals.append(
                jax.core.ShapedArray(tuple(alloc.tensor_shape), mybir.dt.np(alloc.dtype))
            )
    n_params, n_outs = len(in_names), len(out_names)
    all_in_names = tuple(in_names + out_names)
    if partition_name is not None:
        all_in_names = all_in_names + (partition_name,)

    def _body(*args):
        operands = list(args)
        if partition_name is not None:
            operands.append(b2j.partition_id_tensor())
        return tuple(
            b2j._bass_exec_p.bind(
                *operands,
                out_avals=tuple(out_avals),
                in_names=all_in_names,
                out_names=tuple(out_names),
                lowering_input_output_aliases=(),
                sim_require_finite=True,
                sim_require_nnan=True,
                nc=nc,
            )
        )

    devices = jax.devices()[:N_CORES]
    assert len(devices) == N_CORES
    mesh = b2j.Mesh(np.asarray(devices), ("core",))
    in_specs = (b2j.PartitionSpec("core"),) * (n_params + n_outs)
    out_specs = (b2j.PartitionSpec("core"),) * n_outs
    donate = tuple(range(n_params, n_params + n_outs))
    sharded = jax.jit(
        b2j.shard_map(
            _body, mesh=mesh, in_specs=in_specs, out_specs=out_specs, check_rep=False
        ),
        donate_argnums=donate,
        keep_unused=True,
    )
    _CACHE["exec"] = (sharded, in_names, out_names, out_avals, mesh)
    return _CACHE["exec"]


def _stage(y, W1, W2, lambdas):
    """Host prep + device staging. Returns (sharded_fn, dev_inputs, zero_outs)."""
    import jax
    from jax.sharding import NamedSharding, PartitionSpec

    sharded, in_names, out_names, out_avals, mesh = _get_exec()
    in_maps = _prep_inputs(y, W1, W2, lambdas)
    concat_in = [
        np.concatenate([in_maps[c][name] for c in range(N_CORES)], axis=0)
        for name in in_names
    ]
    sh = NamedSharding(mesh, PartitionSpec("core"))
    dev_in = [jax.device_put(a, sh) for a in concat_in]
    zeros = [
        jax.device_put(
            np.zeros((N_CORES * av.shape[0], *av.shape[1:]), av.dtype), sh
        )
        for av in out_avals
    ]
    return sharded, dev_in, zeros, out_avals


def _run(y, W1, W2, lambdas):
    sharded, dev_in, zeros, out_avals = _stage(y, W1, W2, lambdas)
    outs = sharded(*dev_in, *zeros)
    o = np.asarray(outs[0]).reshape(N_CORES, *out_avals[0].shape)
    return _gather([{"out": o[c]} for c in range(N_CORES)])


def kernel(y, W1, W2, lambdas):
    return _run(y, W1, W2, lambdas)


# revision 30
# speedup vs baseline: 43.4242x; 38.1977x over previous
"""Clifford LISTA (nn_CliffordLISTA) Trainium2 Bass kernel.

Math: the Cayley table C[i,j,k] of Cl(3,0) is nonzero only at k = i^j, so
the geometric product einsum 'bni,hnj,ijk->bhk' is, per output blade k:

    out[b,h,k] = sum_{n,i} x[b,n,i] * W[h,n,i^k] * S(i,k),   S(i,k)=C[i,i^k,k]

i.e. each output blade contracts the SAME raw weight tensor W (4MB bf16 for
W2 — it stays resident in SBUF) against x, with a per-(i,k) sign +-1 and a
blade permutation j=i^k that are handled by slice selection at trace time.
The sign is folded into the rhs by keeping a negated copy of x (computed for
free in the epilogue: z-c and c-z).

Feature layout is blade-major (f = blade*H + h) on device so the recurrence
x -> soft(W1y + W2 x) needs zero transposes: xT tiles are
[128 features(part), 256 batch(free)], contraction index is (i, n) blade-
major, and the matmul out^T tile for (blade k, h-chunk) accumulates 32
k-tiles: lhsT = W2sb[:, (j=i^k, nc, hc)-slice], rhs = (+-x)[:, (i,nc)-slice].

Per-blade soft threshold = per-feature clamp: soft(z) = z - clamp(z,-l,+l);
blade is constant within an output tile, so lambda is a [128,1] AP per tile.

Distribution: data-parallel over batch B=2048 across 8 cores (256 each),
weights replicated. HBM traffic per core is only ~11MB total.
"""

import numpy as np
import ml_dtypes

import concourse.bass as bass
import concourse.mybir as mybir
from concourse.tile import TileContext

# ---------------- problem constants (hardcoded per contract) ----------------
DIM = 3
NB = 8                      # blades
B, IN, HID = 2048, 256, 512
N_LAYERS = 5
N_CORES = 8
BL = B // N_CORES           # 256 batch per core
P = 128
NC1 = IN // P               # 2 n-chunks for W1y contraction
NC2 = HID // P              # 4 n-chunks for W2 contraction
K1T = NB * NC1              # 16 k-tiles for W1y
K2T = NB * NC2              # 32 k-tiles for W2 layers
HC = HID // P               # 4 h-chunks per blade
MT = NB * HC                # 32 output feature tiles (blade-major)

_bf16 = ml_dtypes.bfloat16


def _cayley_table(g):
    d = len(g)
    n = 1 << d
    C = np.zeros((n, n, n), dtype=np.float32)
    for a in range(n):
        for b in range(n):
            aa, cnt = a >> 1, 0
            while aa:
                cnt += bin(aa & b).count("1")
                aa >>= 1
            s = -1.0 if (cnt & 1) else 1.0
            for i in range(d):
                if (a >> i) & 1 and (b >> i) & 1:
                    s *= g[i]
            C[a, b, a ^ b] = s
    return C


_C = _cayley_table([1.0, 1.0, 1.0])
_ii = np.arange(NB)[:, None]
_kk = np.arange(NB)[None, :]
_S = _C[_ii, _ii ^ _kk, _kk]         # S[i,k] = C[i, i^k, k], all +-1
_GRADES = np.array([bin(i).count("1") for i in range(NB)])


def _pack_weights(W):
    """[H, N, 8] -> [128, 8*nch*H] bf16, partition-major: per partition p the
    cols are (j, nc, h) blocks with value W[h, nc*128+p, j]."""
    H, N = W.shape[0], W.shape[1]
    ncn = N // P
    T = W.transpose(2, 1, 0).reshape(NB, ncn, P, H).transpose(2, 0, 1, 3)
    return np.ascontiguousarray(T.astype(_bf16)).reshape(P, NB * ncn * H)


# ---------------- Pauli / M2(C) entry-basis machinery (v2) ----------------
# Cl(3,0) ~ M2(C): multivector -> 2x2 complex matrix. Entry channels are
# 2-sparse +-1 combos of blades; the geometric product becomes a 2x2 complex
# matmul = 32 real channel-pair products (vs 64 in blade basis).
# channel order: [00re, 00im, 01re, 01im, 10re, 10im, 11re, 11im]


def _to_entries(v):
    e = np.empty_like(v)
    e[..., 0] = v[..., 0] + v[..., 4]
    e[..., 1] = v[..., 3] + v[..., 7]
    e[..., 2] = v[..., 1] - v[..., 5]
    e[..., 3] = v[..., 6] - v[..., 2]
    e[..., 4] = v[..., 1] + v[..., 5]
    e[..., 5] = v[..., 2] + v[..., 6]
    e[..., 6] = v[..., 0] - v[..., 4]
    e[..., 7] = v[..., 7] - v[..., 3]
    return e


# out-chan e -> 4 terms (a_chan, w_chan, sign); negative-sign w-chans are the
# imaginary channels {1,3,5,7}; their negated copies are stored as weight
# blocks 8 + wc//2.
_TERMS = {
    0: [(0, 0, 1), (1, 1, -1), (2, 4, 1), (3, 5, -1)],
    1: [(0, 1, 1), (1, 0, 1), (2, 5, 1), (3, 4, 1)],
    2: [(0, 2, 1), (1, 3, -1), (2, 6, 1), (3, 7, -1)],
    3: [(0, 3, 1), (1, 2, 1), (2, 7, 1), (3, 6, 1)],
    4: [(4, 0, 1), (5, 1, -1), (6, 4, 1), (7, 5, -1)],
    5: [(4, 1, 1), (5, 0, 1), (6, 5, 1), (7, 4, 1)],
    6: [(4, 2, 1), (5, 3, -1), (6, 6, 1), (7, 7, -1)],
    7: [(4, 3, 1), (5, 2, 1), (6, 7, 1), (7, 6, 1)],
}
_NWB = 12                    # 8 entry channels + 4 negated im channels

# blade k -> (chanA, chanB, sub?) for z_blade = zcA (+/-) zcB (device-halved)
_EXTRACT = {
    0: (0, 6, False), 4: (0, 6, True),
    7: (1, 7, False), 3: (1, 7, True),
    1: (2, 4, False), 5: (4, 2, True),
    6: (5, 3, False), 2: (5, 3, True),
}
# chan e -> (bladeA, bladeB, sub?) for x_chan = xbA (+/-) xbB (true entries)
_REBUILD = {
    0: (0, 4, False), 1: (3, 7, False), 2: (1, 5, True), 3: (6, 2, True),
    4: (1, 5, False), 5: (2, 6, False), 6: (0, 4, True), 7: (7, 3, True),
}


def _pack_weights_pauli(W):
    """[H, N, 8] -> [128, 12*nch*H] bf16: 12 halved entry-channel blocks
    (8 channels + 4 negated im channels), each [nc, p, h] partition-major."""
    H, N = W.shape[0], W.shape[1]
    ncn = N // P
    E = _to_entries(W.astype(np.float64)) * 0.5              # [H, N, 8]
    blocks = [E[..., c] for c in range(8)] + [-E[..., c] for c in (1, 3, 5, 7)]
    E12 = np.stack(blocks, axis=-1)                          # [H, N, 12]
    T = E12.transpose(2, 1, 0).reshape(_NWB, ncn, P, H).transpose(2, 0, 1, 3)
    return np.ascontiguousarray(T.astype(_bf16)).reshape(P, _NWB * ncn * H)


def _build_program(variant="full", reps=1):
    dt = mybir.dt
    nc = bass.Bass()

    y_d = nc.declare_dram_parameter("y", [P, K1T * BL], dt.bfloat16, isOutput=False)
    w1_d = nc.declare_dram_parameter("w1", [P, NB * NC1 * HID], dt.bfloat16, isOutput=False)
    w2_d = nc.declare_dram_parameter("w2", [P, NB * NC2 * HID], dt.bfloat16, isOutput=False)
    lam_d = nc.declare_dram_parameter("lam", [P, 2 * NB], dt.float32, isOutput=False)
    out_d = nc.declare_dram_parameter("out", [MT, P, BL], dt.float32, isOutput=True)

    with TileContext(nc) as tc:
        with (
            tc.tile_pool(name="const", bufs=1) as constp,
            tc.tile_pool(name="state", bufs=1) as statep,
            tc.tile_pool(name="psum", bufs=8, space="PSUM") as ppool,
            tc.tile_pool(name="work", bufs=4) as workp,
        ):
            lam2 = constp.tile([P, 2 * NB], dt.float32, tag="lam2")
            nc.sync.dma_start(out=lam2[:], in_=lam_d[:])
            w2sb = constp.tile([P, NB * NC2 * HID], dt.bfloat16, tag="w2")
            nc.sync.dma_start(out=w2sb[:], in_=w2_d[:])
            w1sb = constp.tile([P, NB * NC1 * HID], dt.bfloat16, tag="w1")
            nc.scalar.dma_start(out=w1sb[:], in_=w1_d[:])
            ysb = constp.tile([P, K1T * BL], dt.bfloat16, tag="y")
            nc.scalar.dma_start(out=ysb[:], in_=y_d[:])
            # negated y for the sign-folded contraction
            ynsb = constp.tile([P, K1T * BL], dt.bfloat16, tag="yn")
            nc.vector.tensor_scalar_mul(ynsb[:], ysb[:], -1.0)

            w1y = statep.tile([P, MT * BL], dt.float32, tag="w1y")
            xp = [
                statep.tile([P, MT * BL], dt.bfloat16, tag=f"x{n}", name=f"x{n}")
                for n in range(2)
            ]
            xn = [
                statep.tile([P, MT * BL], dt.bfloat16, tag=f"xn{n}", name=f"xn{n}")
                for n in range(2)
            ]

            def w1slice(j, ncn, hc):
                base = (j * NC1 + ncn) * HID + hc * P
                return w1sb[:, base:base + P]

            def w2slice(j, ncn, hc):
                base = (j * NC2 + ncn) * HID + hc * P
                return w2sb[:, base:base + P]

            for rep in range(reps):
                # ---- phase 0: W1y (blade-major), save, x1 = soft(W1y) ----
                for k in range(NB):
                    lamp = lam2[:, 2 * k:2 * k + 1]
                    lamn = lam2[:, 2 * k + 1:2 * k + 2]
                    for hc in range(HC):
                        m = k * HC + hc
                        ps = ppool.tile([P, BL], dt.float32, tag="ps")
                        t = 0
                        for i in range(NB):
                            j = i ^ k
                            pos = _S[i, k] > 0
                            for ncn in range(NC1):
                                rhs_t = ysb if pos else ynsb
                                kt = i * NC1 + ncn
                                nc.tensor.matmul(
                                    ps[:],
                                    lhsT=w1slice(j, ncn, hc),
                                    rhs=rhs_t[:, kt * BL:(kt + 1) * BL],
                                    start=(t == 0),
                                    stop=(t == K1T - 1),
                                )
                                t += 1
                        zs = w1y[:, m * BL:(m + 1) * BL]
                        nc.vector.tensor_copy(zs, ps[:])
                        c = workp.tile([P, BL], dt.float32, tag="c")
                        nc.vector.tensor_scalar(
                            c[:], zs, lamp, lamn,
                            op0=mybir.AluOpType.min, op1=mybir.AluOpType.max,
                        )
                        sl = slice(m * BL, (m + 1) * BL)
                        nc.vector.tensor_sub(xp[0][:, sl], zs, c[:])
                        nc.vector.tensor_sub(xn[0][:, sl], c[:], zs)

                # ---- 4 refinement layers ----
                for it in range(N_LAYERS - 1):
                    cur, nxt = it % 2, (it + 1) % 2
                    last = it == N_LAYERS - 2
                    for k in range(NB):
                        lamp = lam2[:, 2 * k:2 * k + 1]
                        lamn = lam2[:, 2 * k + 1:2 * k + 2]
                        for hc in range(HC):
                            m = k * HC + hc
                            ps = ppool.tile([P, BL], dt.float32, tag="ps")
                            t = 0
                            for i in range(NB):
                                j = i ^ k
                                pos = _S[i, k] > 0
                                for ncn in range(NC2):
                                    rhs_t = xp[cur] if pos else xn[cur]
                                    kt = i * NC2 + ncn
                                    nc.tensor.matmul(
                                        ps[:],
                                        lhsT=w2slice(j, ncn, hc),
                                        rhs=rhs_t[:, kt * BL:(kt + 1) * BL],
                                        start=(t == 0),
                                        stop=(t == K2T - 1),
                                    )
                                    t += 1
                            z = workp.tile([P, BL], dt.float32, tag="z")
                            nc.vector.tensor_add(z[:], ps[:], w1y[:, m * BL:(m + 1) * BL])
                            c = workp.tile([P, BL], dt.float32, tag="c")
                            nc.vector.tensor_scalar(
                                c[:], z[:], lamp, lamn,
                                op0=mybir.AluOpType.min, op1=mybir.AluOpType.max,
                            )
                            if last:
                                xo = workp.tile([P, BL], dt.float32, tag="xo")
                                nc.vector.tensor_sub(xo[:], z[:], c[:])
                                nc.sync.dma_start(out=out_d[m], in_=xo[:])
                            else:
                                sl = slice(m * BL, (m + 1) * BL)
                                nc.vector.tensor_sub(xp[nxt][:, sl], z[:], c[:])
                                nc.vector.tensor_sub(xn[nxt][:, sl], c[:], z[:])
    return nc


def _build_program_pauli(variant="full", reps=1):
    """v2: M2(C) entry-basis kernel — 32 channel-pair products per geometric
    product (2x fewer matmuls than blade basis). Threshold done in blade
    space via 2-sparse pairwise combines."""
    dt = mybir.dt
    nc = bass.Bass()

    y_d = nc.declare_dram_parameter("y", [P, K1T * BL], dt.bfloat16, isOutput=False)
    w1_d = nc.declare_dram_parameter("w1", [P, _NWB * NC1 * HID], dt.bfloat16, isOutput=False)
    w2_d = nc.declare_dram_parameter("w2", [P, _NWB * NC2 * HID], dt.bfloat16, isOutput=False)
    lam_d = nc.declare_dram_parameter("lam", [P, 2 * NB], dt.float32, isOutput=False)
    out_d = nc.declare_dram_parameter("out", [MT, P, BL], dt.float32, isOutput=True)

    with TileContext(nc) as tc:
        with (
            tc.tile_pool(name="const", bufs=1) as constp,
            tc.tile_pool(name="state", bufs=1) as statep,
            tc.tile_pool(name="psum", bufs=8, space="PSUM") as ppool,
            tc.tile_pool(name="work", bufs=10) as workp,
        ):
            lam2 = constp.tile([P, 2 * NB], dt.float32, tag="lam2")
            nc.sync.dma_start(out=lam2[:], in_=lam_d[:])
            w2sb = constp.tile([P, _NWB * NC2 * HID], dt.bfloat16, tag="w2")
            nc.sync.dma_start(out=w2sb[:], in_=w2_d[:])
            w1sb = constp.tile([P, _NWB * NC1 * HID], dt.bfloat16, tag="w1")
            nc.scalar.dma_start(out=w1sb[:], in_=w1_d[:])
            ysb = constp.tile([P, K1T * BL], dt.bfloat16, tag="y")
            nc.scalar.dma_start(out=ysb[:], in_=y_d[:])

            w1yc = statep.tile([P, MT * BL], dt.float32, tag="w1yc")
            xc = [
                statep.tile([P, MT * BL], dt.bfloat16, tag=f"xc{n}", name=f"xc{n}")
                for n in range(2)
            ]

            def w1s(wb, ncn, hc):
                base = (wb * NC1 + ncn) * HID + hc * P
                return w1sb[:, base:base + P]

            def w2s(wb, ncn, hc):
                base = (wb * NC2 + ncn) * HID + hc * P
                return w2sb[:, base:base + P]

            def threshold_and_rebuild(zcs, hc, x_dst, last):
                # blade extraction (DVE), soft threshold via two ACT relus
                # (soft(z) = relu(z-l) - relu(-z-l)), chan rebuild (DVE)
                xbs = {}
                for k in range(NB):
                    ca, cb_, sub = _EXTRACT[k]
                    zb = workp.tile([P, BL], dt.float32, tag="zb", name=f"zb{k}")
                    if sub:
                        nc.vector.tensor_sub(zb[:], zcs[ca], zcs[cb_])
                    else:
                        nc.vector.tensor_add(zb[:], zcs[ca], zcs[cb_])
                    cb = workp.tile([P, BL], dt.float32, tag="cb", name=f"cb{k}", bufs=4)
                    nc.vector.tensor_scalar(
                        cb[:], zb[:], lam2[:, 2 * k:2 * k + 1], lam2[:, 2 * k + 1:2 * k + 2],
                        op0=mybir.AluOpType.min, op1=mybir.AluOpType.max,
                    )
                    xb = workp.tile([P, BL], dt.float32, tag="xb", name=f"xb{k}")
                    nc.vector.tensor_sub(xb[:], zb[:], cb[:])
                    xbs[k] = xb
                    if last:
                        nc.sync.dma_start(out=out_d[k * HC + hc], in_=xb[:])
                if not last:
                    for e in range(NB):
                        ba, bb, sub = _REBUILD[e]
                        sl = slice((e * HC + hc) * BL, (e * HC + hc + 1) * BL)
                        if sub:
                            nc.vector.tensor_sub(x_dst[:, sl], xbs[ba][:], xbs[bb][:])
                        else:
                            nc.vector.tensor_add(x_dst[:, sl], xbs[ba][:], xbs[bb][:])

            for rep in range(reps):
                # ---- phase 0: W1y in chan basis, save, x1 = soft(W1y) ----
                for hc in range(HC):
                    zcs = {}
                    for e in (0, 6, 1, 7, 2, 4, 5, 3):
                        ps = ppool.tile([P, BL], dt.float32, tag="ps")
                        t = 0
                        for (ac, wc, s) in _TERMS[e]:
                            wb = wc if s > 0 else 8 + wc // 2
                            for ncn in range(NC1):
                                kt = ac * NC1 + ncn
                                nc.tensor.matmul(
                                    ps[:],
                                    lhsT=w1s(wb, ncn, hc),
                                    rhs=ysb[:, kt * BL:(kt + 1) * BL],
                                    start=(t == 0),
                                    stop=(t == 4 * NC1 - 1),
                                )
                                t += 1
                        zs = w1yc[:, (e * HC + hc) * BL:(e * HC + hc + 1) * BL]
                        nc.vector.tensor_copy(zs, ps[:])
                        zcs[e] = zs
                    threshold_and_rebuild(zcs, hc, xc[0], False)

                # ---- 4 refinement layers ----
                for it in range(N_LAYERS - 1):
                    cur, nxt = it % 2, (it + 1) % 2
                    last = it == N_LAYERS - 2
                    for hc in range(HC):
                        zcs = {}
                        for e in (0, 6, 1, 7, 2, 4, 5, 3):
                            ps = ppool.tile([P, BL], dt.float32, tag="ps")
                            t = 0
                            for ncn in range(NC2):
                                for (ac, wc, s) in _TERMS[e]:
                                    wb = wc if s > 0 else 8 + wc // 2
                                    kt = ac * NC2 + ncn
                                    nc.tensor.matmul(
                                        ps[:],
                                        lhsT=w2s(wb, ncn, hc),
                                        rhs=xc[cur][:, kt * BL:(kt + 1) * BL],
                                        start=(t == 0),
                                        stop=(t == 4 * NC2 - 1),
                                    )
                                    t += 1
                            zc = workp.tile([P, BL], dt.float32, tag="zc", name=f"zc{e}")
                            nc.vector.tensor_add(
                                zc[:], ps[:],
                                w1yc[:, (e * HC + hc) * BL:(e * HC + hc + 1) * BL],
                            )
                            zcs[e] = zc[:]
                        threshold_and_rebuild(zcs, hc, xc[nxt], last)
    return nc


def _split_multi_waits(m):
    """The walrus in this image packs exactly one sync-wait slot per ISA
    instruction; Tile emits several. Hoist the extras onto standalone
    EventSemaphore instructions on the same engine immediately before the
    instruction (identical semantics: all waits gate the same program point).
    """
    for f in m.functions:
        for blk in f.blocks:
            out = []
            for inst in blk.instructions:
                si = inst.sync_info
                if si is not None and si.on_wait is not None and len(si.on_wait) > 1:
                    waits = list(si.on_wait)
                    for j, w in enumerate(waits[:-1]):
                        out.append(
                            mybir.InstEventSemaphore(
                                name=f"{inst.name}-w{j}",
                                opcode="EventSemaphore",
                                engine=inst.engine,
                                ins=[],
                                outs=[],
                                sync_info=mybir.SyncInfo(on_wait=[w], on_update=[]),
                            )
                        )
                    si.on_wait = [waits[-1]]
                out.append(inst)
            blk.instructions = out


_CACHE = {}

VERSION = "pauli"            # "blade" (v1) or "pauli" (v2)


def _prep_inputs_pauli(y, W1, W2, lambdas):
    w1_t = _pack_weights_pauli(np.asarray(W1, dtype=np.float32))
    w2_t = _pack_weights_pauli(np.asarray(W2, dtype=np.float32))

    lam_blade = np.asarray(lambdas, dtype=np.float32)[_GRADES]
    lam2 = np.zeros((P, 2 * NB), np.float32)
    lam2[:, 0::2] = lam_blade[None, :]
    lam2[:, 1::2] = -lam_blade[None, :]

    Yent = _to_entries(np.asarray(y, dtype=np.float32))      # [B, IN, 8]
    in_maps = []
    for cid in range(N_CORES):
        Yc = Yent[cid * BL:(cid + 1) * BL]                   # [256, 256, 8]
        yT = Yc.transpose(2, 1, 0).reshape(NB, NC1, P, BL)   # [ac, nc, p, b]
        yT = yT.transpose(2, 0, 1, 3).reshape(P, K1T * BL)
        yT = np.ascontiguousarray(yT.astype(_bf16))
        in_maps.append({"y": yT, "w1": w1_t, "w2": w2_t, "lam": lam2})
    return in_maps


def _prep_inputs_blade(y, W1, W2, lambdas):
    w1_t = _pack_weights(np.asarray(W1, dtype=np.float32))   # [128, 8192]
    w2_t = _pack_weights(np.asarray(W2, dtype=np.float32))   # [128, 16384]

    lam_blade = np.asarray(lambdas, dtype=np.float32)[_GRADES]   # [8]
    lam2 = np.zeros((P, 2 * NB), np.float32)
    lam2[:, 0::2] = lam_blade[None, :]
    lam2[:, 1::2] = -lam_blade[None, :]

    # y -> blade-major transposed tiles: part p of k-tile (i,ncn) is
    # y[b, ncn*128+p, i]
    Y = np.asarray(y, dtype=np.float32)                      # [B, IN, 8]
    in_maps = []
    for cid in range(N_CORES):
        Yc = Y[cid * BL:(cid + 1) * BL]                      # [256, 256, 8]
        yT = Yc.transpose(2, 1, 0).reshape(NB, NC1, P, BL)   # [i, nc, p, b]
        yT = yT.transpose(2, 0, 1, 3).reshape(P, K1T * BL)   # [p, (i,nc,b)]
        yT = np.ascontiguousarray(yT.astype(_bf16))
        in_maps.append({"y": yT, "w1": w1_t, "w2": w2_t, "lam": lam2})
    return in_maps


def _prep_inputs(y, W1, W2, lambdas):
    if VERSION == "pauli":
        return _prep_inputs_pauli(y, W1, W2, lambdas)
    return _prep_inputs_blade(y, W1, W2, lambdas)


def _build(variant="full", reps=1):
    if VERSION == "pauli":
        return _build_program_pauli(variant, reps)
    return _build_program(variant, reps)


def _gather(results):
    X = np.empty((B, NB * HID), dtype=np.float32)
    for cid in range(N_CORES):
        o = results[cid]["out"]                              # [32, 128, 256]
        X[cid * BL:(cid + 1) * BL] = o.transpose(2, 0, 1).reshape(BL, NB * HID)
    # blade-major f = k*HID + h  ->  [B, H, blades]
    return np.ascontiguousarray(X.reshape(B, NB, HID).transpose(0, 2, 1))


def _get_exec():
    """Compile (once) and return the sharded PJRT executable for the program.

    Mirrors concourse.bass2jax.run_bass_via_pjrt's multi-core path but keeps
    the jitted callable so repeated executions don't re-trace/re-compile.
    """
    if "exec" in _CACHE:
        return _CACHE["exec"]
    import jax
    from concourse import bass2jax as b2j

    nc = _build()
    _split_multi_waits(nc.m)
    assert nc.dbg_addr is None
    partition_name = nc.partition_id_tensor.name if nc.partition_id_tensor else None

    b2j.install_neuronx_cc_hook()
    in_names, out_names, out_avals = [], [], []
    for alloc in nc.m.functions[0].allocations:
        if not isinstance(alloc, mybir.MemoryLocationSet):
            continue
        name = alloc.memorylocations[0].name
        if alloc.kind == "ExternalInput":
            if name != partition_name:
                in_names.append(name)
        elif alloc.kind == "ExternalOutput":
            out_names.append(name)
            out_avals.append(
                jax.core.ShapedArray(tuple(alloc.tensor_shape), mybir.dt.np(alloc.dtype))
            )
    n_params, n_outs = len(in_names), len(out_names)
    all_in_names = tuple(in_names + out_names)
    if partition_name is not None:
        all_in_names = all_in_names + (partition_name,)

    def _body(*args):
        operands = list(args)
        if partition_name is not None:
            operands.append(b2j.partition_id_tensor())
        return tuple(
            b2j._bass_exec_p.bind(
                *operands,
                out_avals=tuple(out_avals),
                in_names=all_in_names,
                out_names=tuple(out_names),
                lowering_input_output_aliases=(),
                sim_require_finite=True,
                sim_require_nnan=True,
                nc=nc,
            )
        )

    devices = jax.devices()[:N_CORES]
    assert len(devices) == N_CORES
    mesh = b2j.Mesh(np.asarray(devices), ("core",))
    in_specs = (b2j.PartitionSpec("core"),) * (n_params + n_outs)
    out_specs = (b2j.PartitionSpec("core"),) * n_outs
    donate = tuple(range(n_params, n_params + n_outs))
    sharded = jax.jit(
        b2j.shard_map(
            _body, mesh=mesh, in_specs=in_specs, out_specs=out_specs, check_rep=False
        ),
        donate_argnums=donate,
        keep_unused=True,
    )
    _CACHE["exec"] = (sharded, in_names, out_names, out_avals, mesh)
    return _CACHE["exec"]


def _stage(y, W1, W2, lambdas):
    """Host prep + device staging. Returns (sharded_fn, dev_inputs, zero_outs)."""
    import jax
    from jax.sharding import NamedSharding, PartitionSpec

    sharded, in_names, out_names, out_avals, mesh = _get_exec()
    in_maps = _prep_inputs(y, W1, W2, lambdas)
    concat_in = [
        np.concatenate([in_maps[c][name] for c in range(N_CORES)], axis=0)
        for name in in_names
    ]
    sh = NamedSharding(mesh, PartitionSpec("core"))
    dev_in = [jax.device_put(a, sh) for a in concat_in]
    zeros = [
        jax.device_put(
            np.zeros((N_CORES * av.shape[0], *av.shape[1:]), av.dtype), sh
        )
        for av in out_avals
    ]
    return sharded, dev_in, zeros, out_avals


def _run(y, W1, W2, lambdas):
    sharded, dev_in, zeros, out_avals = _stage(y, W1, W2, lambdas)
    outs = sharded(*dev_in, *zeros)
    o = np.asarray(outs[0]).reshape(N_CORES, *out_avals[0].shape)
    return _gather([{"out": o[c]} for c in range(N_CORES)])


def kernel(y, W1, W2, lambdas):
    return _run(y, W1, W2, lambdas)
